# revision 15
# baseline (speedup 1.0000x reference)
"""Multi-head attention (b=2, n=2048, d=1024, h=16) on 8 TRN2 NeuronCores.

Sharding: data-parallel over batch (2) x tensor-parallel over head-groups (4).
Core c handles batch c//4, heads 4*(c%4)..4*(c%4)+3 (channel rows 256*(c%4)..).
Column-parallel QKV. The attention outputs (bf16, [256 ch x 512 tok] per
i-block) are re-sharded channels->tokens with a per-block AllToAll (256 KB per
rank, ~4x less wire than AllGather and ~8x less than reduce-scattering the
projected partials), after which each core runs the full output projection +
bias locally for its own 128-token slice of every block and writes the final
rows directly. The host only reassembles slices and casts bf16->f32.

Matmul operands are bf16 (PE full rate; fp32 PSUM accumulation); softmax
statistics and normalization run in fp32. Host-side prep is layout-only
(slicing/transpose/dtype): the device receives x^T and weight shards
pre-transposed so every matmul operand is already in its natural
(contraction-on-partition) layout.

Scheduling notes (to keep the in-order PE queue dense and the HAM clock
warm): the out-projection of i-block k-1 and the second QKV pair are
interleaved as small "filler" chunks inside the attention j-loops, and the
x^T input DMA is chunked per 512-token block so the first projection matmuls
start early.
"""

import sys
from contextlib import ExitStack

_TRN_REPO = "/opt/trn_rl_repo"
if _TRN_REPO not in sys.path:
    sys.path.insert(0, _TRN_REPO)

import ml_dtypes
import numpy as np

import concourse.bass as bass
import concourse.bacc as bacc
import concourse.tile as tile
from concourse import mybir

F32 = mybir.dt.float32
BF16 = mybir.dt.bfloat16

B = 2          # batch
N = 2048       # tokens
D = 1024       # model dim
H = 16         # heads
HD = D // H    # 64 head dim
N_CORES = 8
GROUPS = [[0, 1, 2, 3], [4, 5, 6, 7]]
HPC = 4        # heads per core
CPC = HPC * HD  # 256 channels per core
BW = 512       # attention i-block width (tokens)


def build_program(n=N):
    assert n % BW == 0
    nj = n // 128           # key tiles
    nblk = n // BW          # i blocks

    nc = bacc.Bacc("TRN2", target_bir_lowering=False, debug=False,
                   num_devices=N_CORES)

    # ---- DRAM I/O (per-core shards, host-prepared, bf16) ----
    xt_d = nc.dram_tensor("xt", [D, n], BF16, kind="ExternalInput").ap()
    wqt_d = nc.dram_tensor("wqt", [D, CPC], BF16, kind="ExternalInput").ap()
    wkt_d = nc.dram_tensor("wkt", [D, CPC], BF16, kind="ExternalInput").ap()
    wvt_d = nc.dram_tensor("wvt", [D, CPC], BF16, kind="ExternalInput").ap()
    wot_d = nc.dram_tensor("wot", [D, D], BF16, kind="ExternalInput").ap()
    bo_d = nc.dram_tensor("bob", [128, D], F32, kind="ExternalInput").ap()
    # this core's 128-token slice of each of the 4 i-blocks
    out_d = nc.dram_tensor("out", [n // 4, D], BF16, kind="ExternalOutput").ap()

    # AllGather staging per i-block: this core's [256 ch x 512 tok] attention
    # output; after the AG, rows [256r : +256] hold rank r's channels ->
    # [1024 ch, 512 tok] in global channel order. Each core then pulls only
    # its own 128 token-columns (cc_rank-offset DMA) for the local outproj.
    opq_d = [nc.dram_tensor(f"opq{k}", [CPC, BW], BF16).ap()
             for k in range(nblk)]
    aoq_d = [nc.dram_tensor(f"aoq{k}", [4 * CPC, BW], BF16).ap()
             for k in range(nblk)]

    with tile.TileContext(nc) as tc, ExitStack() as octx:
        wpool = octx.enter_context(tc.tile_pool(name="wpool", bufs=1))
        qk_pool = octx.enter_context(tc.tile_pool(name="qk", bufs=1))
        v_pool = octx.enter_context(tc.tile_pool(name="vaug", bufs=1))
        o_pool = octx.enter_context(tc.tile_pool(name="opair", bufs=1))
        xt_pool = octx.enter_context(tc.tile_pool(name="xt", bufs=1))
        st_pool = octx.enter_context(tc.tile_pool(name="stp", bufs=8))
        nrm_pool = octx.enter_context(tc.tile_pool(name="nrm", bufs=4))
        pp_pool = octx.enter_context(tc.tile_pool(name="pp", bufs=8))
        ago_pool = octx.enter_context(tc.tile_pool(name="ago", bufs=2))
        # PSUM banks: st 2x[128,1024]f32 = 4, ot 2x[65,512] = 2, mm 2x[128,512] = 2
        mm_ps = octx.enter_context(tc.tile_pool(name="mmps", bufs=2, space="PSUM"))
        st_ps_pool = octx.enter_context(
            tc.tile_pool(name="stps", bufs=2, space="PSUM"))
        ot_ps = octx.enter_context(tc.tile_pool(name="otps", bufs=2, space="PSUM"))

        # rank within the 4-core batch group -> token-column offset (elements)
        rank_off = nc.sync.cc_rank(replica_groups=GROUPS) * 128

        # ---- weights (K/Q first: they gate the first matmuls) ----
        def load_w(name, dram, rows, cols):
            nch = rows // 128
            raw = wpool.tile([128, nch * cols], BF16, tag=name, name=name + "_t")
            nc.sync.dma_start(
                raw[:].rearrange("p (c m) -> p c m", c=nch),
                dram.rearrange("(c p) m -> p c m", p=128))
            return raw[:]

        wkt = load_w("wkt", wkt_d, D, CPC)
        wqt = load_w("wqt", wqt_d, D, CPC)

        # ---- x^T, chunked per 512-token block so QKV ic=0 starts early ----
        xt_sb = [xt_pool.tile([128, n], BF16, tag=f"xtr{ch}", name=f"xtr{ch}")
                 for ch in range(8)]
        for cb in range(n // 512):
            for ch in range(8):
                nc.sync.dma_start(
                    xt_sb[ch][:, 512 * cb: 512 * (cb + 1)],
                    xt_d[128 * ch:128 * (ch + 1), 512 * cb: 512 * (cb + 1)])

        wvt = load_w("wvt", wvt_d, D, CPC)
        wot = load_w("wot", wot_d, D, D)

        bias_sb = wpool.tile([128, D], F32, tag="bias")
        nc.sync.dma_start(bias_sb[:], bo_d[:])

        ones_f = wpool.tile([128, 64], F32, tag="ones_f")
        nc.gpsimd.memset(ones_f[:], 1.0)
        ones1 = wpool.tile([1, 64], BF16, tag="ones1")
        nc.vector.tensor_copy(ones1[:], ones_f[0:1, :])

        qtp = [qk_pool.tile([128, n], BF16, tag=f"qtp{p}", name=f"qtp{p}")
               for p in range(2)]
        ktp = [qk_pool.tile([128, n], BF16, tag=f"ktp{p}", name=f"ktp{p}")
               for p in range(2)]
        vaug = [v_pool.tile([128, HPC * 65], BF16, tag=f"vaug{j}", name=f"vaug{j}")
                for j in range(nj)]
        opair = [o_pool.tile([128, n], BF16, tag=f"op{p}", name=f"op{p}")
                 for p in range(2)]

        def qkv_chunk(p, wmat, dst, ic):
            ps = mm_ps.tile([128, 512], F32, tag="mm")
            for ch in range(8):
                nc.tensor.matmul(
                    ps[:],
                    wmat[:, ch * 256 + p * 128: ch * 256 + p * 128 + 128],
                    xt_sb[ch][:, 512 * ic: 512 * (ic + 1)],
                    start=(ch == 0), stop=(ch == 7))
            nc.vector.tensor_copy(dst[p][:, 512 * ic: 512 * (ic + 1)], ps[:])

        def qkv_pair(p):
            for (wmat, dst) in ((wkt, ktp), (wqt, qtp)):
                for ic in range(n // 512):
                    qkv_chunk(p, wmat, dst, ic)

        def v_phase():
            for j in range(nj):
                nc.vector.tensor_copy(
                    vaug[j][:].rearrange("p (h m) -> p h m", h=HPC)[:, :, 64:65],
                    ones_f[:].rearrange("p (h m) -> p h m", m=1)[:, 0:HPC, :])
                for half in range(2):
                    ps = mm_ps.tile([128, 512], F32, tag="mm")
                    for ch in range(8):
                        nc.tensor.matmul(
                            ps[:, 0:128],
                            xt_sb[ch][:, 128 * j: 128 * (j + 1)],
                            wvt[:, ch * 256 + 128 * half:
                                ch * 256 + 128 * half + 128],
                            start=(ch == 0), stop=(ch == 7))
                    dst = vaug[j][:].rearrange(
                        "p (h m) -> p h m", h=HPC)[:, 2 * half: 2 * half + 2, 0:64]
                    src = ps[:, 0:128].rearrange("p (h m) -> p h m", h=2)
                    nc.vector.tensor_copy(dst, src)

        scale = float(HD) ** -0.5

        def attn_block(p, ib, fillers=None, fill_start=0):
            """Heads 2p,2p+1 for i-block ib. Scores for both heads land in one
            [128,1024] PSUM tile (head-even cols 0-511, head-odd 512-1023) so a
            single FD=1024 exp serves both. `fillers` is a list of callables
            (out-projection / QKV chunks) drained one per j iteration starting
            at j=fill_start to keep the PE queue dense while exps are in
            flight. After normalization the block's opair columns are staged
            to DRAM for the AllToAll."""
            i0 = ib * BW
            fillers = list(fillers) if fillers else []
            fi = 0
            ots = [ot_ps.tile([65, BW], F32, tag="ot", name=f"ot{p}_{ib}_{e}")
                   for e in range(2)]
            def emit_av(j, st_sb):
                for e in range(2):
                    nc.tensor.matmul(
                        ots[e][:],
                        vaug[j][:, 65 * (2 * p + e): 65 * (2 * p + e) + 65],
                        st_sb[:, 512 * e: 512 * e + 512],
                        start=(j == 0), stop=(j == nj - 1))

            # AV emitted 2 iterations behind scores/exp so the in-order PE
            # never head-of-line blocks waiting for the current exp.
            pend = []
            for j in range(nj):
                st_ps = st_ps_pool.tile([128, 1024], F32, tag="st")
                for e in range(2):
                    r0 = 64 * e
                    nc.tensor.matmul(
                        st_ps[:, 512 * e: 512 * e + 512],
                        ktp[p][r0:r0 + 64, 128 * j: 128 * (j + 1)],
                        qtp[p][r0:r0 + 64, i0: i0 + BW],
                        start=True, stop=True)
                st_sb = st_pool.tile([128, 1024], BF16, tag="st")
                nc.scalar.activation(
                    st_sb[:], st_ps[:],
                    mybir.ActivationFunctionType.Exp, scale=scale)
                if fi < len(fillers) and j >= fill_start:
                    fillers[fi]()
                    fi += 1
                pend.append((j, st_sb))
                if len(pend) > 2:
                    emit_av(*pend.pop(0))
            for item in pend:
                emit_av(*item)
            while fi < len(fillers):
                fillers[fi]()
                fi += 1
            # softmax normalization: denominator row -> reciprocal -> bf16 ->
            # broadcast down 64 partitions via a tiny ones-stationary matmul
            # -> scale the AV block into opair.
            for e in range(2):
                dsb = nrm_pool.tile([1, BW], F32, tag="dsb")
                nc.vector.tensor_copy(dsb[:], ots[e][64:65, :])
                rsb = nrm_pool.tile([1, BW], F32, tag="rsb")
                nc.vector.reciprocal_approx_fast(rsb[:], dsb[:])
                rsr = nrm_pool.tile([1, BW], BF16, tag="rsr")
                nc.vector.tensor_copy(rsr[:], rsb[:])
                bps = mm_ps.tile([128, 512], F32, tag="mm")
                nc.tensor.matmul(bps[0:64, :], ones1[:], rsr[:],
                                 start=True, stop=True)
                bsb = nrm_pool.tile([64, BW], F32, tag="bsb")
                nc.vector.tensor_copy(bsb[:], bps[0:64, :])
                nc.vector.tensor_mul(
                    opair[p][64 * e: 64 * e + 64, i0: i0 + BW],
                    ots[e][0:64, :], bsb[:])
            # stage this head-pair's channels for the AllGather
            nc.sync.dma_start(
                opq_d[ib][128 * p: 128 * (p + 1), :],
                opair[p][:, i0: i0 + BW])

        def post_ag(k):
            """Gather the group's attention-output channels for block k, then
            pull this core's own 128 token-columns (rank-offset DMA) into
            SBUF for the local out-projection."""
            nc.gpsimd.collective_compute(
                "AllGather", mybir.AluOpType.bypass, replica_groups=GROUPS,
                ins=[opq_d[k][:]], outs=[aoq_d[k][:]])
            ago = ago_pool.tile([128, 8 * 128], BF16, tag="ago",
                                name=f"ago{k}")
            src = aoq_d[k].rearrange("(c p) m -> p c m", p=128)[:, :, 0:128]
            src = bass.AP(src.tensor, src.offset + rank_off, src.ap)
            nc.sync.dma_start(
                ago[:].rearrange("p (c m) -> p c m", c=8), src)
            return ago

        def outproj_chunks(k, ago):
            """Local out-projection of this core's 128 tokens of block k:
            full 1024-channel contraction + full bias, straight to out_d.
            4 filler chunks."""
            chunks = []
            for oc in range(2):
                ps_box = []
                def chunk_a(oc=oc, ps_box=ps_box):
                    ps = mm_ps.tile([128, 512], F32, tag="mm")
                    ps_box.append(ps)
                    for c in range(4):
                        nc.tensor.matmul(
                            ps[:], ago[:, 128 * c: 128 * (c + 1)],
                            wot[:, 1024 * c + 512 * oc: 1024 * c + 512 * oc + 512],
                            start=(c == 0), stop=False)
                def chunk_b(k=k, oc=oc, ps_box=ps_box):
                    ps = ps_box[0]
                    for c in range(4, 8):
                        nc.tensor.matmul(
                            ps[:], ago[:, 128 * c: 128 * (c + 1)],
                            wot[:, 1024 * c + 512 * oc: 1024 * c + 512 * oc + 512],
                            start=False, stop=(c == 7))
                    pp_sb = pp_pool.tile([128, 512], BF16, tag="pp")
                    nc.vector.tensor_add(
                        pp_sb[:], ps[:], bias_sb[:, 512 * oc: 512 * oc + 512])
                    nc.sync.dma_start(
                        out_d[128 * k: 128 * (k + 1), 512 * oc: 512 * oc + 512],
                        pp_sb[:])
                chunks += [chunk_a, chunk_b]
            return chunks

        # ---- schedule: QKV p1 inside block 0; A2A(k) posted right after
        # block k's staging DMAs; outproj(k) fillers inside block k+1 ----
        qkv_pair(0)
        v_phase()
        qkv1 = [lambda p=p, w=w, d=d, ic=ic: qkv_chunk(p, w, d, ic)
                for (w, d) in ((wkt, ktp), (wqt, qtp)) for ic in range(n // 512)
                for p in (1,)]
        attn_block(0, 0, fillers=qkv1)
        attn_block(1, 0)
        ago = post_ag(0)
        for k in range(1, nblk):
            attn_block(0, k, fillers=outproj_chunks(k - 1, ago), fill_start=4)
            attn_block(1, k)
            ago = post_ag(k)
        for chunk in outproj_chunks(nblk - 1, ago):
            chunk()

    nc.compile()
    return nc


def make_in_maps(x, wq, wk, wv, wo, bo):
    """Host-side sharding + layout prep (slices/transposes/dtype only)."""
    bf = ml_dtypes.bfloat16
    x = np.asarray(x, dtype=np.float32)
    bo_b = np.ascontiguousarray(
        np.broadcast_to(np.asarray(bo, np.float32)[None, :], (128, D)))
    wq, wk, wv, wo = (np.asarray(w, np.float32) for w in (wq, wk, wv, wo))
    wot = np.ascontiguousarray(wo.T.astype(bf))
    in_maps = []
    for c in range(N_CORES):
        b, g = divmod(c, 4)
        r0 = CPC * g
        in_maps.append({
            "xt": np.ascontiguousarray(x[b].T.astype(bf)),
            "wqt": np.ascontiguousarray(wq[r0:r0 + CPC, :].T.astype(bf)),
            "wkt": np.ascontiguousarray(wk[r0:r0 + CPC, :].T.astype(bf)),
            "wvt": np.ascontiguousarray(wv[r0:r0 + CPC, :].T.astype(bf)),
            "wot": wot,
            "bob": bo_b,
        })
    return in_maps


_PROG_CACHE = {}


def _get_prog(n=N):
    if n not in _PROG_CACHE:
        _PROG_CACHE[n] = build_program(n)
    return _PROG_CACHE[n]


def run(x, wq, wk, wv, wo, bo, trace=False, trace_cores=None):
    """Run on hardware; returns (output [B,N,D], exec_time_ns or None)."""
    from concourse.bass_utils import run_bass_kernel_spmd

    nc = _get_prog()
    in_maps = make_in_maps(x, wq, wk, wv, wo, bo)
    kw = {}
    if trace:
        kw = dict(trace=True, trace_cores=trace_cores or [0])
    res = run_bass_kernel_spmd(nc, in_maps, list(range(N_CORES)), **kw)
    out = np.empty((B, N, D), dtype=np.float32)
    nblk = N // BW
    for c in range(N_CORES):
        b, g = divmod(c, 4)
        o = np.asarray(res.results[c]["out"], dtype=np.float32)
        for k in range(nblk):
            t0 = BW * k + 128 * g
            out[b, t0:t0 + 128, :] = o[128 * k: 128 * (k + 1)]
    return out, res.exec_time_ns


def kernel(x, wq, wk, wv, wo, bo):
    out, _ = run(x, wq, wk, wv, wo, bo)
    return out


# revision 18
# speedup vs baseline: 1.1157x; 1.1157x over previous
"""Multi-head attention (b=2, n=2048, d=1024, h=16) on 8 TRN2 NeuronCores.

Sharding: data-parallel over batch (2) x tensor-parallel over head-groups (4).
Core c handles batch c//4, heads 4*(c%4)..4*(c%4)+3 (channel rows 256*(c%4)..).
Column-parallel QKV. The attention outputs (bf16, [256 ch x 512 tok] per
i-block) are re-sharded channels->tokens with a per-block AllToAll (256 KB per
rank, ~4x less wire than AllGather and ~8x less than reduce-scattering the
projected partials), after which each core runs the full output projection +
bias locally for its own 128-token slice of every block and writes the final
rows directly. The host only reassembles slices and casts bf16->f32.

Matmul operands are bf16 (PE full rate; fp32 PSUM accumulation); softmax
statistics and normalization run in fp32. Host-side prep is layout-only
(slicing/transpose/dtype): the device receives x^T and weight shards
pre-transposed so every matmul operand is already in its natural
(contraction-on-partition) layout.

Scheduling notes (to keep the in-order PE queue dense and the HAM clock
warm): the out-projection of i-block k-1 and the second QKV pair are
interleaved as small "filler" chunks inside the attention j-loops, and the
x^T input DMA is chunked per 512-token block so the first projection matmuls
start early.
"""

import sys
from contextlib import ExitStack

_TRN_REPO = "/opt/trn_rl_repo"
if _TRN_REPO not in sys.path:
    sys.path.insert(0, _TRN_REPO)

import ml_dtypes
import numpy as np

import concourse.bass as bass
import concourse.bacc as bacc
import concourse.tile as tile
from concourse import mybir

F32 = mybir.dt.float32
BF16 = mybir.dt.bfloat16

B = 2          # batch
N = 2048       # tokens
D = 1024       # model dim
H = 16         # heads
HD = D // H    # 64 head dim
N_CORES = 8
GROUPS = [[0, 1, 2, 3], [4, 5, 6, 7]]
HPC = 4        # heads per core
CPC = HPC * HD  # 256 channels per core
BW = 512       # attention i-block width (tokens)


def build_program(n=N):
    assert n % BW == 0
    nj = n // 128           # key tiles
    nblk = n // BW          # i blocks

    nc = bacc.Bacc("TRN2", target_bir_lowering=False, debug=False,
                   num_devices=N_CORES)

    # ---- DRAM I/O (per-core shards, host-prepared, bf16) ----
    xt_d = nc.dram_tensor("xt", [D, n], BF16, kind="ExternalInput").ap()
    wqt_d = nc.dram_tensor("wqt", [D, CPC], BF16, kind="ExternalInput").ap()
    wkt_d = nc.dram_tensor("wkt", [D, CPC], BF16, kind="ExternalInput").ap()
    wvt_d = nc.dram_tensor("wvt", [D, CPC], BF16, kind="ExternalInput").ap()
    wot_d = nc.dram_tensor("wot", [D, D], BF16, kind="ExternalInput").ap()
    bo_d = nc.dram_tensor("bob", [128, D], F32, kind="ExternalInput").ap()
    # this core's 128-token slice of each of the 4 i-blocks
    out_d = nc.dram_tensor("out", [n // 4, D], BF16, kind="ExternalOutput").ap()

    # AllGather staging per (i-block, head-pair): this core's [128 ch x 512
    # tok] half of the attention output; after the AG, rows [128r : +128]
    # hold rank r's p-half channels (global channel chunk 2r+p). Posting per
    # head-pair starts the collective half a block earlier and lets the tail
    # out-projection begin accumulating on the p0 half. Each core then pulls
    # only its own 128 token-columns (cc_rank-offset DMA).
    opq_d = [[nc.dram_tensor(f"opq{k}_{p}", [128, BW], BF16).ap()
              for p in range(2)] for k in range(nblk)]
    aoq_d = [[nc.dram_tensor(f"aoq{k}_{p}", [512, BW], BF16).ap()
              for p in range(2)] for k in range(nblk)]

    with tile.TileContext(nc) as tc, ExitStack() as octx:
        wpool = octx.enter_context(tc.tile_pool(name="wpool", bufs=1))
        qk_pool = octx.enter_context(tc.tile_pool(name="qk", bufs=1))
        v_pool = octx.enter_context(tc.tile_pool(name="vaug", bufs=1))
        o_pool = octx.enter_context(tc.tile_pool(name="opair", bufs=1))
        xt_pool = octx.enter_context(tc.tile_pool(name="xt", bufs=1))
        st_pool = octx.enter_context(tc.tile_pool(name="stp", bufs=8))
        nrm_pool = octx.enter_context(tc.tile_pool(name="nrm", bufs=4))
        pp_pool = octx.enter_context(tc.tile_pool(name="pp", bufs=8))
        ago_pool = octx.enter_context(tc.tile_pool(name="ago", bufs=2))
        # PSUM banks: st 2x[128,1024]f32 = 4, ot 2x[65,512] = 2, mm 2x[128,512] = 2
        mm_ps = octx.enter_context(tc.tile_pool(name="mmps", bufs=2, space="PSUM"))
        st_ps_pool = octx.enter_context(
            tc.tile_pool(name="stps", bufs=2, space="PSUM"))
        ot_ps = octx.enter_context(tc.tile_pool(name="otps", bufs=2, space="PSUM"))

        # rank within the 4-core batch group -> token-column offset (elements)
        rank_off = nc.sync.cc_rank(replica_groups=GROUPS) * 128

        # ---- weights (K/Q first: they gate the first matmuls) ----
        def load_w(name, dram, rows, cols):
            nch = rows // 128
            raw = wpool.tile([128, nch * cols], BF16, tag=name, name=name + "_t")
            nc.sync.dma_start(
                raw[:].rearrange("p (c m) -> p c m", c=nch),
                dram.rearrange("(c p) m -> p c m", p=128))
            return raw[:]

        wkt = load_w("wkt", wkt_d, D, CPC)
        wqt = load_w("wqt", wqt_d, D, CPC)

        # ---- x^T, chunked per 512-token block so QKV ic=0 starts early ----
        xt_sb = [xt_pool.tile([128, n], BF16, tag=f"xtr{ch}", name=f"xtr{ch}")
                 for ch in range(8)]
        for cb in range(n // 512):
            for ch in range(8):
                nc.sync.dma_start(
                    xt_sb[ch][:, 512 * cb: 512 * (cb + 1)],
                    xt_d[128 * ch:128 * (ch + 1), 512 * cb: 512 * (cb + 1)])

        wvt = load_w("wvt", wvt_d, D, CPC)
        wot = load_w("wot", wot_d, D, D)

        bias_sb = wpool.tile([128, D], F32, tag="bias")
        nc.sync.dma_start(bias_sb[:], bo_d[:])

        ones_f = wpool.tile([128, 64], F32, tag="ones_f")
        nc.gpsimd.memset(ones_f[:], 1.0)
        ones1 = wpool.tile([1, 64], BF16, tag="ones1")
        nc.vector.tensor_copy(ones1[:], ones_f[0:1, :])

        qtp = [qk_pool.tile([128, n], BF16, tag=f"qtp{p}", name=f"qtp{p}")
               for p in range(2)]
        ktp = [qk_pool.tile([128, n], BF16, tag=f"ktp{p}", name=f"ktp{p}")
               for p in range(2)]
        vaug = [v_pool.tile([128, HPC * 65], BF16, tag=f"vaug{j}", name=f"vaug{j}")
                for j in range(nj)]
        opair = [o_pool.tile([128, n], BF16, tag=f"op{p}", name=f"op{p}")
                 for p in range(2)]

        def qkv_chunk(p, wmat, dst, ic):
            ps = mm_ps.tile([128, 512], F32, tag="mm")
            for ch in range(8):
                nc.tensor.matmul(
                    ps[:],
                    wmat[:, ch * 256 + p * 128: ch * 256 + p * 128 + 128],
                    xt_sb[ch][:, 512 * ic: 512 * (ic + 1)],
                    start=(ch == 0), stop=(ch == 7))
            nc.vector.tensor_copy(dst[p][:, 512 * ic: 512 * (ic + 1)], ps[:])

        def qkv_pair(p):
            for (wmat, dst) in ((wkt, ktp), (wqt, qtp)):
                for ic in range(n // 512):
                    qkv_chunk(p, wmat, dst, ic)

        def v_phase():
            for j in range(nj):
                nc.vector.tensor_copy(
                    vaug[j][:].rearrange("p (h m) -> p h m", h=HPC)[:, :, 64:65],
                    ones_f[:].rearrange("p (h m) -> p h m", m=1)[:, 0:HPC, :])
                for half in range(2):
                    ps = mm_ps.tile([128, 512], F32, tag="mm")
                    for ch in range(8):
                        nc.tensor.matmul(
                            ps[:, 0:128],
                            xt_sb[ch][:, 128 * j: 128 * (j + 1)],
                            wvt[:, ch * 256 + 128 * half:
                                ch * 256 + 128 * half + 128],
                            start=(ch == 0), stop=(ch == 7))
                    dst = vaug[j][:].rearrange(
                        "p (h m) -> p h m", h=HPC)[:, 2 * half: 2 * half + 2, 0:64]
                    src = ps[:, 0:128].rearrange("p (h m) -> p h m", h=2)
                    nc.vector.tensor_copy(dst, src)

        scale = float(HD) ** -0.5

        def attn_block(p, ib, fillers=None, fill_start=0):
            """Heads 2p,2p+1 for i-block ib. Scores for both heads land in one
            [128,1024] PSUM tile (head-even cols 0-511, head-odd 512-1023) so a
            single FD=1024 exp serves both. `fillers` is a list of callables
            (out-projection / QKV chunks) drained one per j iteration starting
            at j=fill_start to keep the PE queue dense while exps are in
            flight. After normalization the block's opair columns are staged
            to DRAM for the AllToAll."""
            i0 = ib * BW
            fillers = list(fillers) if fillers else []
            fi = 0
            ots = [ot_ps.tile([65, BW], F32, tag="ot", name=f"ot{p}_{ib}_{e}")
                   for e in range(2)]
            def emit_av(j, st_sb):
                for e in range(2):
                    nc.tensor.matmul(
                        ots[e][:],
                        vaug[j][:, 65 * (2 * p + e): 65 * (2 * p + e) + 65],
                        st_sb[:, 512 * e: 512 * e + 512],
                        start=(j == 0), stop=(j == nj - 1))

            # AV emitted 2 iterations behind scores/exp so the in-order PE
            # never head-of-line blocks waiting for the current exp.
            pend = []
            for j in range(nj):
                st_ps = st_ps_pool.tile([128, 1024], F32, tag="st")
                for e in range(2):
                    r0 = 64 * e
                    nc.tensor.matmul(
                        st_ps[:, 512 * e: 512 * e + 512],
                        ktp[p][r0:r0 + 64, 128 * j: 128 * (j + 1)],
                        qtp[p][r0:r0 + 64, i0: i0 + BW],
                        start=True, stop=True)
                st_sb = st_pool.tile([128, 1024], BF16, tag="st")
                nc.scalar.activation(
                    st_sb[:], st_ps[:],
                    mybir.ActivationFunctionType.Exp, scale=scale)
                if fi < len(fillers) and j >= fill_start:
                    fillers[fi]()
                    fi += 1
                pend.append((j, st_sb))
                if len(pend) > 2:
                    emit_av(*pend.pop(0))
            for item in pend:
                emit_av(*item)
            while fi < len(fillers):
                fillers[fi]()
                fi += 1
            # softmax normalization: denominator row -> reciprocal -> bf16 ->
            # broadcast down 64 partitions via a tiny ones-stationary matmul
            # -> scale the AV block into opair.
            for e in range(2):
                dsb = nrm_pool.tile([1, BW], F32, tag="dsb")
                nc.vector.tensor_copy(dsb[:], ots[e][64:65, :])
                rsb = nrm_pool.tile([1, BW], F32, tag="rsb")
                nc.vector.reciprocal_approx_fast(rsb[:], dsb[:])
                rsr = nrm_pool.tile([1, BW], BF16, tag="rsr")
                nc.vector.tensor_copy(rsr[:], rsb[:])
                bps = mm_ps.tile([128, 512], F32, tag="mm")
                nc.tensor.matmul(bps[0:64, :], ones1[:], rsr[:],
                                 start=True, stop=True)
                bsb = nrm_pool.tile([64, BW], F32, tag="bsb")
                nc.vector.tensor_copy(bsb[:], bps[0:64, :])
                nc.vector.tensor_mul(
                    opair[p][64 * e: 64 * e + 64, i0: i0 + BW],
                    ots[e][0:64, :], bsb[:])
        ago_tiles = {}

        def stage_ag(k, p):
            """Stage head-pair p's channels of block k, AllGather them across
            the group, and pull this core's own 128 token-columns
            (rank-offset DMA) into its half of the block's ago tile: cols
            [512p : 512p+512] hold chunks for global channel groups 2r+p."""
            nc.sync.dma_start(opq_d[k][p][:], opair[p][:, BW * k: BW * (k + 1)])
            nc.gpsimd.collective_compute(
                "AllGather", mybir.AluOpType.bypass, replica_groups=GROUPS,
                ins=[opq_d[k][p][:]], outs=[aoq_d[k][p][:]])
            if k not in ago_tiles:
                ago_tiles[k] = ago_pool.tile([128, 8 * 128], BF16, tag="ago",
                                             name=f"ago{k}")
            ago = ago_tiles[k]
            src = aoq_d[k][p].rearrange("(c q) m -> q c m", q=128)[:, :, 0:128]
            src = bass.AP(src.tensor, src.offset + rank_off, src.ap)
            nc.sync.dma_start(
                ago[:, 512 * p: 512 * (p + 1)].rearrange(
                    "q (c m) -> q c m", c=4), src)
            return ago

        def outproj_chunks(k, ago):
            """Local out-projection of this core's 128 tokens of block k:
            full 1024-channel contraction + full bias, straight to out_d.
            ago col-chunk c maps to global channel group 2c (p0 half, c<4)
            or 2(c-4)+1 (p1 half). chunk_a only needs the p0 AllGather."""
            chunks = []
            for oc in range(2):
                ps_box = []
                def chunk_a(oc=oc, ps_box=ps_box):
                    ps = mm_ps.tile([128, 512], F32, tag="mm")
                    ps_box.append(ps)
                    for c in range(4):
                        g = 2 * c
                        nc.tensor.matmul(
                            ps[:], ago[:, 128 * c: 128 * (c + 1)],
                            wot[:, 1024 * g + 512 * oc: 1024 * g + 512 * oc + 512],
                            start=(c == 0), stop=False)
                def chunk_b(k=k, oc=oc, ps_box=ps_box):
                    ps = ps_box[0]
                    for c in range(4, 8):
                        g = 2 * (c - 4) + 1
                        nc.tensor.matmul(
                            ps[:], ago[:, 128 * c: 128 * (c + 1)],
                            wot[:, 1024 * g + 512 * oc: 1024 * g + 512 * oc + 512],
                            start=False, stop=(c == 7))
                    pp_sb = pp_pool.tile([128, 512], BF16, tag="pp")
                    nc.vector.tensor_add(
                        pp_sb[:], ps[:], bias_sb[:, 512 * oc: 512 * oc + 512])
                    nc.sync.dma_start(
                        out_d[128 * k: 128 * (k + 1), 512 * oc: 512 * oc + 512],
                        pp_sb[:])
                chunks += [chunk_a, chunk_b]
            return chunks

        # ---- schedule: QKV p1 inside block 0; AllGather (k, p) posted right
        # after head-pair p's normalization; outproj(k) fillers inside
        # attn1(k+1), a full half-block after the last AG post ----
        qkv_pair(0)
        v_phase()
        qkv1 = [lambda p=p, w=w, d=d, ic=ic: qkv_chunk(p, w, d, ic)
                for (w, d) in ((wkt, ktp), (wqt, qtp)) for ic in range(n // 512)
                for p in (1,)]
        attn_block(0, 0, fillers=qkv1)
        stage_ag(0, 0)
        attn_block(1, 0)
        ago = stage_ag(0, 1)
        for k in range(1, nblk):
            attn_block(0, k)
            stage_ag(k, 0)
            attn_block(1, k, fillers=outproj_chunks(k - 1, ago), fill_start=2)
            ago = stage_ag(k, 1)
        for chunk in outproj_chunks(nblk - 1, ago):
            chunk()

    nc.compile()
    return nc


def make_in_maps(x, wq, wk, wv, wo, bo):
    """Host-side sharding + layout prep (slices/transposes/dtype only)."""
    bf = ml_dtypes.bfloat16
    x = np.asarray(x, dtype=np.float32)
    bo_b = np.ascontiguousarray(
        np.broadcast_to(np.asarray(bo, np.float32)[None, :], (128, D)))
    wq, wk, wv, wo = (np.asarray(w, np.float32) for w in (wq, wk, wv, wo))
    wot = np.ascontiguousarray(wo.T.astype(bf))
    in_maps = []
    for c in range(N_CORES):
        b, g = divmod(c, 4)
        r0 = CPC * g
        in_maps.append({
            "xt": np.ascontiguousarray(x[b].T.astype(bf)),
            "wqt": np.ascontiguousarray(wq[r0:r0 + CPC, :].T.astype(bf)),
            "wkt": np.ascontiguousarray(wk[r0:r0 + CPC, :].T.astype(bf)),
            "wvt": np.ascontiguousarray(wv[r0:r0 + CPC, :].T.astype(bf)),
            "wot": wot,
            "bob": bo_b,
        })
    return in_maps


_PROG_CACHE = {}


def _get_prog(n=N):
    if n not in _PROG_CACHE:
        _PROG_CACHE[n] = build_program(n)
    return _PROG_CACHE[n]


def run(x, wq, wk, wv, wo, bo, trace=False, trace_cores=None):
    """Run on hardware; returns (output [B,N,D], exec_time_ns or None)."""
    from concourse.bass_utils import run_bass_kernel_spmd

    nc = _get_prog()
    in_maps = make_in_maps(x, wq, wk, wv, wo, bo)
    kw = {}
    if trace:
        kw = dict(trace=True, trace_cores=trace_cores or [0])
    res = run_bass_kernel_spmd(nc, in_maps, list(range(N_CORES)), **kw)
    out = np.empty((B, N, D), dtype=np.float32)
    nblk = N // BW
    for c in range(N_CORES):
        b, g = divmod(c, 4)
        o = np.asarray(res.results[c]["out"], dtype=np.float32)
        for k in range(nblk):
            t0 = BW * k + 128 * g
            out[b, t0:t0 + 128, :] = o[128 * k: 128 * (k + 1)]
    return out, res.exec_time_ns


def kernel(x, wq, wk, wv, wo, bo):
    out, _ = run(x, wq, wk, wv, wo, bo)
    return out
